# revision 32
# baseline (speedup 1.0000x reference)
"""Multi-head attention (B=2, S=2048, D=1024, H=16) on 8 TRN2 NeuronCores.

Sharding: tensor-parallel on heads (2 heads = 128 channels per core).
Everything on-device runs in "transposed" layout [channel, B*S]:
  - host passes hiddenT [D, B*S] (bf16) replicated to all cores
  - per-core Q/K/V projections produce qT/kT/vT [128, B*S]
  - attention per (batch, 512-query chunk), BOTH heads together:
      per key tile the two heads' score matmuls contract over disjoint
      64-row groups of the PE array (h0 rows 0-63, h1 rows 64-127) so
      they execute CONCURRENTLY (row tiling); they write the two halves
      of one [128,1024] PSUM tile, which a single ScalarE exp (mask as
      per-partition bias, 1/sqrt(hd) as scale) turns into pr=[h0|h1].
      PV matmuls accumulate per-head ctx [65,512] (ones row = softmax
      denominator). Normalization is FUSED into ctx evacuation:
      tensor_mul reads ctx straight from PSUM against the reciprocal
      broadcast (sel-matmul + reciprocal_approx_fast).
  - per-core partial output projection outT[o, n] += Wo[o, own 128
    chans] @ ctxn (bf16 out); host reduces the 8 partials in f32.

Phase emission order keeps TensorE fed with "filler" matmuls
(projections for the other batch, output projection for the previous
batch) inside the ScalarE-bound attention inner loop.
"""

import numpy as np
import ml_dtypes

import concourse.bass as bass
import concourse.mybir as mybir
import concourse.tile as tile
from concourse import bacc
from concourse import bass_utils
from concourse.masks import make_identity

F32 = mybir.dt.float32
BF16 = mybir.dt.bfloat16
BF16_NP = ml_dtypes.bfloat16

B, S, D, H = 2, 2048, 1024, 16
HD = D // H
BS = B * S            # 4096
P = 128               # partitions / channels per core
NCORES = 8
KT = S // P           # 16 key tiles per batch
NQ = 512              # matmul moving free dim
VA_W = HD + 1         # v_aug columns per key tile (64 v cols + ones col)
QC = 512              # attention query-chunk width
NC = S // QC          # 4 query chunks per batch

_CACHE = {}


def _build():
    nc = bacc.Bacc("TRN2", target_bir_lowering=False, debug=False,
                   num_devices=NCORES)

    hT = nc.dram_tensor("hT", [D, BS], BF16, kind="ExternalInput")
    wq = nc.dram_tensor("wq", [D, P], BF16, kind="ExternalInput")
    wk = nc.dram_tensor("wk", [D, P], BF16, kind="ExternalInput")
    wv = nc.dram_tensor("wv", [D, P], BF16, kind="ExternalInput")
    wo = nc.dram_tensor("wo", [P, D], BF16, kind="ExternalInput")
    bq = nc.dram_tensor("bq", [P, 1], F32, kind="ExternalInput")
    bk = nc.dram_tensor("bk", [P, 1], F32, kind="ExternalInput")
    bv = nc.dram_tensor("bv", [P, 1], F32, kind="ExternalInput")
    bo = nc.dram_tensor("bo", [P, 1], F32, kind="ExternalInput")
    maskT = nc.dram_tensor("maskT", [S, B], F32, kind="ExternalInput")
    sel = nc.dram_tensor("sel", [33, P], BF16, kind="ExternalInput")
    outT = nc.dram_tensor("outT", [D, BS], BF16, kind="ExternalOutput")

    with tile.TileContext(nc) as tc:
        with (
            tc.tile_pool(name="const", bufs=1) as const,
            tc.tile_pool(name="res", bufs=1) as res,
            tc.tile_pool(name="ht", bufs=32) as ht_pool,
            tc.tile_pool(name="va", bufs=2) as va_pool,
            tc.tile_pool(name="pr", bufs=10) as pr_pool,
            tc.tile_pool(name="bc", bufs=2) as bc_pool,
            tc.tile_pool(name="ot", bufs=3) as ot_pool,
            # PSUM: pj 1x[128,512](1 bank) + po 1x[128,512](1) +
            #       sc 2x[128,1024](4) + ctx 2tags x[65,512](2) = 8 banks
            tc.tile_pool(name="pj_ps", bufs=1, space="PSUM") as pj_ps,
            tc.tile_pool(name="po_ps", bufs=1, space="PSUM") as po_ps,
            tc.tile_pool(name="sc_ps", bufs=2, space="PSUM") as sc_ps,
            tc.tile_pool(name="ctx_ps", bufs=1, space="PSUM") as ctx_ps,
        ):
            # ---- startup: wk + biases first so the first k-proj matmul
            # can begin after ~1.5us of DMA; wq/wv stream in behind it ----
            w_sbs = {}
            t = const.tile([P, D], BF16, name="wk_sb", tag="wk_sb")
            nc.sync.dma_start(
                t[:].rearrange("p (j m) -> p j m", j=D // P),
                wk.ap().rearrange("(j p) m -> p j m", p=P))
            w_sbs["wk"] = t
            b_sbs = {}
            for nm, bt in (("bk", bk), ("bq", bq), ("bv", bv), ("bo", bo)):
                t = const.tile([P, 1], F32, name=f"{nm}_sb", tag=f"{nm}_sb")
                nc.sync.dma_start(t[:], bt.ap())
                b_sbs[nm] = t
            # warm the ScalarE exp table during startup DMA
            dummy = const.tile([1, 1], BF16)
            nc.scalar.activation(dummy[:], b_sbs["bk"][0:1, 0:1],
                                 mybir.ActivationFunctionType.Exp)

            qT = res.tile([P, BS], BF16)
            kT = res.tile([P, BS], BF16)
            vT = res.tile([P, BS], BF16)
            ctxn = res.tile([P, BS], BF16)
            # softmax sums: h0 at partition 0, h1 at partition 32 (engine
            # ops need 32-aligned partition bases); rows 1-31 are zeroed
            # once so the sel-matmul contraction over rows 0-32 is clean
            s2_sb = res.tile([33, BS], BF16)
            nc.vector.memset(s2_sb[:], 0.0)

            VA = {}

            def setup_va(b):
                vas = []
                for h in range(2):
                    va = va_pool.tile([P, KT * VA_W], BF16, name=f"va{b}{h}",
                                      tag=f"va{h}")
                    nc.vector.memset(va[:], 1.0)
                    vas.append(va)
                VA[b] = vas

            def proj_va_steps(b, nlo, nhi):
                """Projections + v_aug build for 512-col chunks [nlo,nhi) of
                batch b as a generator of small emission steps (PE filler
                inside attention). k first: attention QKs gate on kT."""
                vas = VA[b]
                boff = b * S
                for n in range(b * 4 + nlo, b * 4 + nhi):
                    nsl = bass.ts(n, NQ)
                    hts = []
                    for k in range(D // P):
                        htt = ht_pool.tile([P, NQ], BF16, name=f"ht{k}",
                                           tag="ht")
                        nc.sync.dma_start(htt[:], hT.ap()[bass.ts(k, P), nsl])
                        hts.append(htt)
                    yield
                    for wn, bn, dest in (("wk", "bk", kT), ("wq", "bq", qT),
                                         ("wv", "bv", vT)):
                        ps = pj_ps.tile([P, NQ], F32, name=f"ps_{wn}",
                                        tag="pj")
                        for k in range(D // P):
                            nc.tensor.matmul(
                                ps[:], w_sbs[wn][:, bass.ts(k, P)], hts[k][:],
                                start=(k == 0), stop=(k == D // P - 1))
                            if k % 4 == 3:
                                yield
                        nc.vector.tensor_scalar_add(
                            dest[:, nsl], ps[:], b_sbs[bn][:])
                        yield
                    # vT for this 512-col chunk is done -> its 4 key tiles
                    # can be transposed into v_aug
                    nlocal = n - b * 4
                    for kt in range(nlocal * 4, nlocal * 4 + 4):
                        for h in range(2):
                            hsl = slice(h * HD, (h + 1) * HD)
                            tp = pj_ps.tile([P, HD], BF16, name="tp",
                                            tag="pj")
                            nc.tensor.transpose(
                                tp[:],
                                vT[hsl, boff + kt * P:boff + (kt + 1) * P],
                                ident[hsl, :])
                            nc.vector.tensor_copy(
                                vas[h][:, kt * VA_W:kt * VA_W + HD], tp[:])
                        yield

            CTX = {}

            def attn_chunk(b, c, filler, quiet_head=0, quiet_tail=0,
                           pops=1, lag=4):
                """Both heads' attention for one 512-col query chunk.
                Per key tile: the 2 heads' score matmuls land on disjoint
                64-row groups (concurrent), one exp covers both heads,
                2 PV matmuls accumulate per-head ctx (ones row = sums)."""
                va0, va1 = VA[b]
                boff = b * S
                coff = boff + c * QC
                ctx0 = ctx_ps.tile([HD + 1, QC], F32, name=f"ctx{b}{c}0",
                                   tag="ctx0")
                ctx1 = ctx_ps.tile([HD + 1, QC], F32, name=f"ctx{b}{c}1",
                                   tag="ctx1")
                CTX[(b, c)] = (ctx0, ctx1)
                # PV trails the score/exp stream by `lag` key tiles so the
                # first PV of this chunk (which blocks on the PREVIOUS
                # chunk's ctx release via its gather chain) never stalls
                # the in-order PE stream ahead of the exps.
                prs = {}

                def pv(kt):
                    pr = prs.pop(kt)
                    nc.tensor.matmul(
                        ctx0[:], va0[:, kt * VA_W:(kt + 1) * VA_W],
                        pr[:, 0:QC], start=(kt == 0), stop=(kt == KT - 1))
                    nc.tensor.matmul(
                        ctx1[:], va1[:, kt * VA_W:(kt + 1) * VA_W],
                        pr[:, QC:2 * QC], start=(kt == 0), stop=(kt == KT - 1))

                for kt in range(KT):
                    if filler is not None and quiet_head <= kt < KT - quiet_tail:
                        for _ in range(pops):
                            next(filler, None)
                    sct = sc_ps.tile([P, 2 * QC], F32, name="sct", tag="sct")
                    ksl = slice(boff + kt * P, boff + (kt + 1) * P)
                    nc.tensor.matmul(
                        sct[:, 0:QC], kT[0:HD, ksl], qT[0:HD, coff:coff + QC],
                        start=True, stop=True)
                    nc.tensor.matmul(
                        sct[:, QC:2 * QC], kT[HD:P, ksl],
                        qT[HD:P, coff:coff + QC], start=True, stop=True)
                    pr = pr_pool.tile([P, 2 * QC], BF16, name="pr", tag="pr")
                    prs[kt] = pr
                    nc.scalar.activation(
                        pr[:], sct[:], mybir.ActivationFunctionType.Exp,
                        bias=mask_sb[:, b * KT + kt:b * KT + kt + 1],
                        scale=0.125)
                    if kt >= lag:
                        pv(kt - lag)
                for kt in range(KT - lag, KT):
                    pv(kt)

            def gather_norm(b, c):
                """normalize ctx for chunk (b, c) straight out of PSUM:
                sums -> SBUF, sel-matmul broadcast, reciprocal, then one
                fused tensor_mul per head. high_priority so the chain
                threads in as soon as its deps resolve."""
                with tc.high_priority():
                    ctx0, ctx1 = CTX.pop((b, c))
                    goff = b * S + c * QC
                    # sum rows via ScalarE: it idles at the chunk boundary
                    # (next scores sit behind the PV flush on the PE), and
                    # VectorE's shorter remaining chain releases ctx for
                    # the next chunk's PV sooner
                    nc.scalar.activation(
                        s2_sb[0:1, goff:goff + QC], ctx0[HD:HD + 1, :],
                        mybir.ActivationFunctionType.Copy, bias=0.0)
                    nc.scalar.activation(
                        s2_sb[32:33, goff:goff + QC], ctx1[HD:HD + 1, :],
                        mybir.ActivationFunctionType.Copy, bias=0.0)
                    pbc = po_ps.tile([P, QC], F32, name="pbc", tag="po")
                    nc.tensor.matmul(pbc[:], sel_sb[:],
                                     s2_sb[:, goff:goff + QC],
                                     start=True, stop=True)
                    bcr = bc_pool.tile([P, QC], F32, name="bcr", tag="bcr")
                    nc.vector.reciprocal_approx_fast(bcr[:], pbc[:])
                    nc.vector.tensor_mul(
                        ctxn[0:HD, goff:goff + QC], ctx0[0:HD, :],
                        bcr[0:HD, :])
                    nc.vector.tensor_mul(
                        ctxn[HD:P, goff:goff + QC], ctx1[0:HD, :],
                        bcr[HD:P, :])

            def oproj_steps(b, clo=0, chi=NC, evac_act=False):
                """partial output projection for batch b: outT[o, n] +=
                Wo[o, own chans] @ ctxn — full o range, own 128 channels;
                the cross-core reduction happens on the host (bo too)"""
                boff = b * S
                for cg in range(clo, chi):
                    goff = boff + cg * QC
                    for t in range(D // P):
                        pool = po_ps if t % 2 == 0 else pj_ps
                        po = pool.tile([P, QC], F32, name="po",
                                       tag="pj" if t % 2 else "po")
                        nc.tensor.matmul(
                            po[:], wo_sb[:, bass.ts(t, P)],
                            ctxn[:, goff:goff + QC],
                            start=True, stop=True)
                        ot = ot_pool.tile([P, QC], BF16, name="ot", tag="ot")
                        if evac_act:
                            nc.scalar.activation(
                                ot[:], po[:],
                                mybir.ActivationFunctionType.Copy, bias=0.0)
                        else:
                            nc.vector.tensor_copy(ot[:], po[:])
                        # alternate DMA paths: HWDGE (sync) / SWDGE (gpsimd)
                        eng = nc.sync if t % 2 == 0 else nc.gpsimd
                        eng.dma_start(
                            outT.ap()[bass.ts(t, P), goff:goff + QC], ot[:])
                        if t % 2 == 1:
                            yield
                    yield

            def oproj_tail(b, cg):
                """last output-projection chunk: evacuation alternating
                ScalarE/VectorE (both idle post-attention) and DMA
                alternating sync/gpsimd — keeps the MM->evac->DMA drain
                chain dense and parallel so the kernel tail is short."""
                goff = b * S + cg * QC
                for t in range(D // P):
                    pool = po_ps if t % 2 == 0 else pj_ps
                    po = pool.tile([P, QC], F32, name="po",
                                   tag="pj" if t % 2 else "po")
                    nc.tensor.matmul(
                        po[:], wo_sb[:, bass.ts(t, P)],
                        ctxn[:, goff:goff + QC], start=True, stop=True)
                    ot = ot_pool.tile([P, QC], BF16, name="ot", tag="ot")
                    if t % 2 == 0:
                        nc.scalar.activation(
                            ot[:], po[:],
                            mybir.ActivationFunctionType.Copy, bias=0.0)
                    else:
                        nc.vector.tensor_copy(ot[:], po[:])
                    eng = nc.sync if t % 2 == 0 else nc.gpsimd
                    eng.dma_start(
                        outT.ap()[bass.ts(t, P), goff:goff + QC], ot[:])

            def drain(g):
                for _ in g:
                    pass

            # ---- software pipeline ----
            setup_va(0)
            g0 = proj_va_steps(0, 0, 1)
            next(g0)                   # emit ht chunk-0 DMAs right away
            ident = const.tile([P, HD], BF16)
            make_identity(nc, ident[0:HD, :])
            # stream the remaining weights/constants behind the ht tiles
            for nm, w in (("wq", wq), ("wv", wv)):
                t = const.tile([P, D], BF16, name=f"{nm}_sb", tag=f"{nm}_sb")
                nc.sync.dma_start(
                    t[:].rearrange("p (j m) -> p j m", j=D // P),
                    w.ap().rearrange("(j p) m -> p j m", p=P))
                w_sbs[nm] = t
            mask_sb = const.tile([P, B * KT], F32)
            nc.sync.dma_start(
                mask_sb[:].rearrange("p (b t) -> p b t", b=B),
                maskT.ap().rearrange("(t p) b -> p b t", p=P))
            nc.sync.dma_start(ident[HD:P, :], ident[0:HD, :])
            drain(g0)                  # finish b0 chunk 0 up front
            wo_sb = const.tile([P, D], BF16)
            nc.sync.dma_start(wo_sb[:], wo.ap())
            sel_sb = const.tile([33, P], BF16)
            nc.sync.dma_start(sel_sb[:], sel.ap())

            fA0 = proj_va_steps(0, 1, 4)
            attn_chunk(0, 0, fA0, pops=4)
            drain(fA0)
            setup_va(1)
            # one continuous filler stream across all remaining attention
            # chunks: b1 projections, then the output projections as their
            # gathers land. chain() evaluates lazily, so each oproj
            # generator is only emitted after its gather exists.
            import itertools
            F = itertools.chain(
                proj_va_steps(1, 0, 4),
                oproj_steps(0, 0, 4),
                oproj_steps(1, 0, 2),
            )
            attn_chunk(0, 1, F, quiet_head=1, quiet_tail=2)
            gather_norm(0, 0)
            attn_chunk(0, 2, F, quiet_head=1, quiet_tail=2, pops=2)
            gather_norm(0, 1)
            attn_chunk(0, 3, F, quiet_head=1, quiet_tail=2)
            gather_norm(0, 2)
            gather_norm(0, 3)
            # batch boundary: ctx(1,0) can only allocate once gather(0,3)
            # finishes, so run this chunk's PV an extra-deep lag behind
            attn_chunk(1, 0, F, quiet_head=1, quiet_tail=2, lag=8)
            gather_norm(1, 0)
            attn_chunk(1, 1, F, quiet_head=1, quiet_tail=2)
            gather_norm(1, 1)
            attn_chunk(1, 2, F, quiet_head=1, quiet_tail=2)
            drain(F)
            gather_norm(1, 2)
            fD = oproj_steps(1, 2, 3)
            attn_chunk(1, 3, fD, quiet_head=1, quiet_tail=3)
            drain(fD)
            gather_norm(1, 3)
            oproj_tail(1, 3)

    nc.compile()
    return nc


def _prep_inputs(hidden_state, attention_mask, Wq, bq, Wk, bk, Wv, bv, Wo, bo):
    h2 = np.ascontiguousarray(
        np.asarray(hidden_state, dtype=np.float32).reshape(BS, D).T
    ).astype(BF16_NP)
    maskT = np.ascontiguousarray(
        np.asarray(attention_mask, dtype=np.float32).reshape(B, S).T)
    selm = np.zeros((33, P), dtype=BF16_NP)
    selm[0, 0:HD] = 1
    selm[32, HD:P] = 1
    in_maps = []
    for c in range(NCORES):
        sl = slice(c * P, (c + 1) * P)
        in_maps.append({
            "hT": h2,
            "wq": np.ascontiguousarray(np.asarray(Wq)[sl, :].T).astype(BF16_NP),
            "wk": np.ascontiguousarray(np.asarray(Wk)[sl, :].T).astype(BF16_NP),
            "wv": np.ascontiguousarray(np.asarray(Wv)[sl, :].T).astype(BF16_NP),
            "wo": np.ascontiguousarray(np.asarray(Wo)[:, sl].T).astype(BF16_NP),
            "bq": np.asarray(bq, dtype=np.float32)[sl].reshape(P, 1),
            "bk": np.asarray(bk, dtype=np.float32)[sl].reshape(P, 1),
            "bv": np.asarray(bv, dtype=np.float32)[sl].reshape(P, 1),
            "bo": np.asarray(bo, dtype=np.float32)[sl].reshape(P, 1),
            "maskT": maskT,
            "sel": selm,
        })
    return in_maps


def kernel(**inputs) -> np.ndarray:
    if "nc" not in _CACHE:
        _CACHE["nc"] = _build()
    nc = _CACHE["nc"]
    in_maps = _prep_inputs(**inputs)
    res = bass_utils.run_bass_kernel_spmd(
        nc, in_maps, core_ids=list(range(NCORES)))
    outT = res.results[0]["outT"].astype(np.float32)
    for c in range(1, NCORES):
        outT += res.results[c]["outT"].astype(np.float32)
    out = np.ascontiguousarray(outT.T).reshape(B, S, D)
    out += np.asarray(inputs["bo"], dtype=np.float32)
    return out.astype(np.float32)


# revision 33
# speedup vs baseline: 1.0268x; 1.0268x over previous
"""Multi-head attention (B=2, S=2048, D=1024, H=16) on 8 TRN2 NeuronCores.

Sharding: tensor-parallel on heads (2 heads = 128 channels per core).
Everything on-device runs in "transposed" layout [channel, B*S]:
  - host passes hiddenT [D, B*S] (bf16) replicated to all cores
  - per-core Q/K/V projections produce qT/kT/vT [128, B*S]
  - attention per (batch, 512-query chunk), BOTH heads together:
      per key tile the two heads' score matmuls contract over disjoint
      64-row groups of the PE array (h0 rows 0-63, h1 rows 64-127) so
      they execute CONCURRENTLY (row tiling); they write the two halves
      of one [128,1024] PSUM tile, which a single ScalarE exp (mask as
      per-partition bias, 1/sqrt(hd) as scale) turns into pr=[h0|h1].
      PV matmuls accumulate per-head ctx [65,512] (ones row = softmax
      denominator). Normalization is FUSED into ctx evacuation:
      tensor_mul reads ctx straight from PSUM against the reciprocal
      broadcast (sel-matmul + reciprocal_approx_fast).
  - per-core partial output projection outT[o, n] += Wo[o, own 128
    chans] @ ctxn (bf16 out); host reduces the 8 partials in f32.

Phase emission order keeps TensorE fed with "filler" matmuls
(projections for the other batch, output projection for the previous
batch) inside the ScalarE-bound attention inner loop.
"""

import numpy as np
import ml_dtypes

import concourse.bass as bass
import concourse.mybir as mybir
import concourse.tile as tile
from concourse import bacc
from concourse import bass_utils
from concourse.masks import make_identity

F32 = mybir.dt.float32
BF16 = mybir.dt.bfloat16
BF16_NP = ml_dtypes.bfloat16

B, S, D, H = 2, 2048, 1024, 16
HD = D // H
BS = B * S            # 4096
P = 128               # partitions / channels per core
NCORES = 8
KT = S // P           # 16 key tiles per batch
NQ = 512              # matmul moving free dim
VA_W = HD + 1         # v_aug columns per key tile (64 v cols + ones col)
QC = 512              # attention query-chunk width
NC = S // QC          # 4 query chunks per batch

_CACHE = {}


def _build():
    nc = bacc.Bacc("TRN2", target_bir_lowering=False, debug=False,
                   num_devices=NCORES)

    hT = nc.dram_tensor("hT", [D, BS], BF16, kind="ExternalInput")
    wq = nc.dram_tensor("wq", [D, P], BF16, kind="ExternalInput")
    wk = nc.dram_tensor("wk", [D, P], BF16, kind="ExternalInput")
    wv = nc.dram_tensor("wv", [D, P], BF16, kind="ExternalInput")
    wo = nc.dram_tensor("wo", [P, D], BF16, kind="ExternalInput")
    bq = nc.dram_tensor("bq", [P, 1], F32, kind="ExternalInput")
    bk = nc.dram_tensor("bk", [P, 1], F32, kind="ExternalInput")
    bv = nc.dram_tensor("bv", [P, 1], F32, kind="ExternalInput")
    bo = nc.dram_tensor("bo", [P, 1], F32, kind="ExternalInput")
    maskT = nc.dram_tensor("maskT", [S, B], F32, kind="ExternalInput")
    sel = nc.dram_tensor("sel", [33, P], BF16, kind="ExternalInput")
    outT = nc.dram_tensor("outT", [D, BS], BF16, kind="ExternalOutput")

    with tile.TileContext(nc) as tc:
        with (
            tc.tile_pool(name="const", bufs=1) as const,
            tc.tile_pool(name="res", bufs=1) as res,
            tc.tile_pool(name="ht", bufs=32) as ht_pool,
            tc.tile_pool(name="va", bufs=2) as va_pool,
            tc.tile_pool(name="pr", bufs=10) as pr_pool,
            tc.tile_pool(name="bc", bufs=2) as bc_pool,
            tc.tile_pool(name="ot", bufs=3) as ot_pool,
            # PSUM: pj 1x[128,512](1 bank) + po 1x[128,512](1) +
            #       sc 2x[128,1024](4) + ctx 2tags x[65,512](2) = 8 banks
            tc.tile_pool(name="pj_ps", bufs=1, space="PSUM") as pj_ps,
            tc.tile_pool(name="po_ps", bufs=1, space="PSUM") as po_ps,
            tc.tile_pool(name="sc_ps", bufs=2, space="PSUM") as sc_ps,
            tc.tile_pool(name="ctx_ps", bufs=1, space="PSUM") as ctx_ps,
        ):
            # ---- startup: wk + biases first so the first k-proj matmul
            # can begin after ~1.5us of DMA; wq/wv stream in behind it ----
            w_sbs = {}
            t = const.tile([P, D], BF16, name="wk_sb", tag="wk_sb")
            nc.sync.dma_start(
                t[:].rearrange("p (j m) -> p j m", j=D // P),
                wk.ap().rearrange("(j p) m -> p j m", p=P))
            w_sbs["wk"] = t
            b_sbs = {}
            for nm, bt in (("bk", bk), ("bq", bq), ("bv", bv), ("bo", bo)):
                t = const.tile([P, 1], F32, name=f"{nm}_sb", tag=f"{nm}_sb")
                nc.sync.dma_start(t[:], bt.ap())
                b_sbs[nm] = t
            # warm the ScalarE exp table during startup DMA
            dummy = const.tile([1, 1], BF16)
            nc.scalar.activation(dummy[:], b_sbs["bk"][0:1, 0:1],
                                 mybir.ActivationFunctionType.Exp)

            qT = res.tile([P, BS], BF16)
            kT = res.tile([P, BS], BF16)
            vT = res.tile([P, BS], BF16)
            ctxn = res.tile([P, BS], BF16)
            # softmax sums: h0 at partition 0, h1 at partition 32 (engine
            # ops need 32-aligned partition bases); rows 1-31 are zeroed
            # once so the sel-matmul contraction over rows 0-32 is clean
            s2_sb = res.tile([33, BS], BF16)
            nc.vector.memset(s2_sb[:], 0.0)

            VA = {}

            def setup_va(b):
                vas = []
                for h in range(2):
                    va = va_pool.tile([P, KT * VA_W], BF16, name=f"va{b}{h}",
                                      tag=f"va{h}")
                    nc.vector.memset(va[:], 1.0)
                    vas.append(va)
                VA[b] = vas

            def proj_va_steps(b, nlo, nhi):
                """Projections + v_aug build for 512-col chunks [nlo,nhi) of
                batch b as a generator of small emission steps (PE filler
                inside attention). k first: attention QKs gate on kT."""
                vas = VA[b]
                boff = b * S
                for n in range(b * 4 + nlo, b * 4 + nhi):
                    nsl = bass.ts(n, NQ)
                    hts = []
                    for k in range(D // P):
                        htt = ht_pool.tile([P, NQ], BF16, name=f"ht{k}",
                                           tag="ht")
                        nc.sync.dma_start(htt[:], hT.ap()[bass.ts(k, P), nsl])
                        hts.append(htt)
                    yield
                    for wn, bn, dest in (("wk", "bk", kT), ("wq", "bq", qT),
                                         ("wv", "bv", vT)):
                        ps = pj_ps.tile([P, NQ], F32, name=f"ps_{wn}",
                                        tag="pj")
                        for k in range(D // P):
                            nc.tensor.matmul(
                                ps[:], w_sbs[wn][:, bass.ts(k, P)], hts[k][:],
                                start=(k == 0), stop=(k == D // P - 1))
                            if k % 4 == 3:
                                yield
                        nc.vector.tensor_scalar_add(
                            dest[:, nsl], ps[:], b_sbs[bn][:])
                        yield
                    # vT for this 512-col chunk is done -> its 4 key tiles
                    # can be transposed into v_aug
                    nlocal = n - b * 4
                    for kt in range(nlocal * 4, nlocal * 4 + 4):
                        for h in range(2):
                            hsl = slice(h * HD, (h + 1) * HD)
                            tp = pj_ps.tile([P, HD], BF16, name="tp",
                                            tag="pj")
                            nc.tensor.transpose(
                                tp[:],
                                vT[hsl, boff + kt * P:boff + (kt + 1) * P],
                                ident[hsl, :])
                            nc.vector.tensor_copy(
                                vas[h][:, kt * VA_W:kt * VA_W + HD], tp[:])
                        yield

            CTX = {}

            def attn_chunk(b, c, filler, quiet_head=0, quiet_tail=0,
                           pops=1, lag=4):
                """Both heads' attention for one 512-col query chunk.
                Per key tile: the 2 heads' score matmuls land on disjoint
                64-row groups (concurrent), one exp covers both heads,
                2 PV matmuls accumulate per-head ctx (ones row = sums)."""
                va0, va1 = VA[b]
                boff = b * S
                coff = boff + c * QC
                ctx0 = ctx_ps.tile([HD + 1, QC], F32, name=f"ctx{b}{c}0",
                                   tag="ctx0")
                ctx1 = ctx_ps.tile([HD + 1, QC], F32, name=f"ctx{b}{c}1",
                                   tag="ctx1")
                CTX[(b, c)] = (ctx0, ctx1)
                # PV trails the score/exp stream by `lag` key tiles so the
                # first PV of this chunk (which blocks on the PREVIOUS
                # chunk's ctx release via its gather chain) never stalls
                # the in-order PE stream ahead of the exps.
                prs = {}

                def pv(kt):
                    pr = prs.pop(kt)
                    nc.tensor.matmul(
                        ctx0[:], va0[:, kt * VA_W:(kt + 1) * VA_W],
                        pr[:, 0:QC], start=(kt == 0), stop=(kt == KT - 1))
                    nc.tensor.matmul(
                        ctx1[:], va1[:, kt * VA_W:(kt + 1) * VA_W],
                        pr[:, QC:2 * QC], start=(kt == 0), stop=(kt == KT - 1))

                for kt in range(KT):
                    if filler is not None and quiet_head <= kt < KT - quiet_tail:
                        for _ in range(pops):
                            next(filler, None)
                    sct = sc_ps.tile([P, 2 * QC], F32, name="sct", tag="sct")
                    ksl = slice(boff + kt * P, boff + (kt + 1) * P)
                    nc.tensor.matmul(
                        sct[:, 0:QC], kT[0:HD, ksl], qT[0:HD, coff:coff + QC],
                        start=True, stop=True)
                    nc.tensor.matmul(
                        sct[:, QC:2 * QC], kT[HD:P, ksl],
                        qT[HD:P, coff:coff + QC], start=True, stop=True)
                    pr = pr_pool.tile([P, 2 * QC], BF16, name="pr", tag="pr")
                    prs[kt] = pr
                    nc.scalar.activation(
                        pr[:], sct[:], mybir.ActivationFunctionType.Exp,
                        bias=mask_sb[:, b * KT + kt:b * KT + kt + 1],
                        scale=0.125)
                    if kt >= lag:
                        pv(kt - lag)
                for kt in range(KT - lag, KT):
                    pv(kt)

            def gather_norm(b, c):
                """normalize ctx for chunk (b, c) straight out of PSUM:
                sums -> SBUF, sel-matmul broadcast, reciprocal, then one
                fused tensor_mul per head. high_priority so the chain
                threads in as soon as its deps resolve."""
                with tc.high_priority():
                    ctx0, ctx1 = CTX.pop((b, c))
                    goff = b * S + c * QC
                    nc.vector.tensor_copy(s2_sb[0:1, goff:goff + QC],
                                          ctx0[HD:HD + 1, :])
                    nc.vector.tensor_copy(s2_sb[32:33, goff:goff + QC],
                                          ctx1[HD:HD + 1, :])
                    pbc = po_ps.tile([P, QC], F32, name="pbc", tag="po")
                    nc.tensor.matmul(pbc[:], sel_sb[:],
                                     s2_sb[:, goff:goff + QC],
                                     start=True, stop=True)
                    bcr = bc_pool.tile([P, QC], F32, name="bcr", tag="bcr")
                    nc.vector.reciprocal_approx_fast(bcr[:], pbc[:])
                    nc.vector.tensor_mul(
                        ctxn[0:HD, goff:goff + QC], ctx0[0:HD, :],
                        bcr[0:HD, :])
                    nc.vector.tensor_mul(
                        ctxn[HD:P, goff:goff + QC], ctx1[0:HD, :],
                        bcr[HD:P, :])

            def oproj_steps(b, clo=0, chi=NC, evac_act=False):
                """partial output projection for batch b: outT[o, n] +=
                Wo[o, own chans] @ ctxn — full o range, own 128 channels;
                the cross-core reduction happens on the host (bo too)"""
                boff = b * S
                for cg in range(clo, chi):
                    goff = boff + cg * QC
                    for t in range(D // P):
                        pool = po_ps if t % 2 == 0 else pj_ps
                        po = pool.tile([P, QC], F32, name="po",
                                       tag="pj" if t % 2 else "po")
                        nc.tensor.matmul(
                            po[:], wo_sb[:, bass.ts(t, P)],
                            ctxn[:, goff:goff + QC],
                            start=True, stop=True)
                        ot = ot_pool.tile([P, QC], BF16, name="ot", tag="ot")
                        if evac_act:
                            nc.scalar.activation(
                                ot[:], po[:],
                                mybir.ActivationFunctionType.Copy, bias=0.0)
                        else:
                            nc.vector.tensor_copy(ot[:], po[:])
                        # alternate DMA paths: HWDGE (sync) / SWDGE (gpsimd)
                        eng = nc.sync if t % 2 == 0 else nc.gpsimd
                        eng.dma_start(
                            outT.ap()[bass.ts(t, P), goff:goff + QC], ot[:])
                        if t % 2 == 1:
                            yield
                    yield

            def oproj_tail(b, cg):
                """last output-projection chunk: evacuation alternating
                ScalarE/VectorE (both idle post-attention) and DMA
                alternating sync/gpsimd — keeps the MM->evac->DMA drain
                chain dense and parallel so the kernel tail is short."""
                goff = b * S + cg * QC
                for t in range(D // P):
                    pool = po_ps if t % 2 == 0 else pj_ps
                    po = pool.tile([P, QC], F32, name="po",
                                   tag="pj" if t % 2 else "po")
                    nc.tensor.matmul(
                        po[:], wo_sb[:, bass.ts(t, P)],
                        ctxn[:, goff:goff + QC], start=True, stop=True)
                    ot = ot_pool.tile([P, QC], BF16, name="ot", tag="ot")
                    if t % 2 == 0:
                        nc.scalar.activation(
                            ot[:], po[:],
                            mybir.ActivationFunctionType.Copy, bias=0.0)
                    else:
                        nc.vector.tensor_copy(ot[:], po[:])
                    eng = nc.sync if t % 2 == 0 else nc.gpsimd
                    eng.dma_start(
                        outT.ap()[bass.ts(t, P), goff:goff + QC], ot[:])

            def drain(g):
                for _ in g:
                    pass

            # ---- software pipeline ----
            setup_va(0)
            g0 = proj_va_steps(0, 0, 1)
            next(g0)                   # emit ht chunk-0 DMAs right away
            ident = const.tile([P, HD], BF16)
            make_identity(nc, ident[0:HD, :])
            # stream the remaining weights/constants behind the ht tiles
            for nm, w in (("wq", wq), ("wv", wv)):
                t = const.tile([P, D], BF16, name=f"{nm}_sb", tag=f"{nm}_sb")
                nc.sync.dma_start(
                    t[:].rearrange("p (j m) -> p j m", j=D // P),
                    w.ap().rearrange("(j p) m -> p j m", p=P))
                w_sbs[nm] = t
            mask_sb = const.tile([P, B * KT], F32)
            nc.sync.dma_start(
                mask_sb[:].rearrange("p (b t) -> p b t", b=B),
                maskT.ap().rearrange("(t p) b -> p b t", p=P))
            nc.sync.dma_start(ident[HD:P, :], ident[0:HD, :])
            drain(g0)                  # finish b0 chunk 0 up front
            wo_sb = const.tile([P, D], BF16)
            nc.sync.dma_start(wo_sb[:], wo.ap())
            sel_sb = const.tile([33, P], BF16)
            nc.sync.dma_start(sel_sb[:], sel.ap())

            fA0 = proj_va_steps(0, 1, 4)
            attn_chunk(0, 0, fA0, pops=4)
            drain(fA0)
            setup_va(1)
            # one continuous filler stream across all remaining attention
            # chunks: b1 projections, then the output projections as their
            # gathers land. chain() evaluates lazily, so each oproj
            # generator is only emitted after its gather exists.
            import itertools
            F = itertools.chain(
                proj_va_steps(1, 0, 4),
                oproj_steps(0, 0, 4),
                oproj_steps(1, 0, 2),
            )
            attn_chunk(0, 1, F, quiet_head=1, quiet_tail=2)
            gather_norm(0, 0)
            attn_chunk(0, 2, F, quiet_head=1, quiet_tail=2, pops=2)
            gather_norm(0, 1)
            attn_chunk(0, 3, F, quiet_head=1, quiet_tail=2)
            gather_norm(0, 2)
            gather_norm(0, 3)
            # batch boundary: ctx(1,0) can only allocate once gather(0,3)
            # finishes, so run this chunk's PV an extra-deep lag behind
            attn_chunk(1, 0, F, quiet_head=1, quiet_tail=2, lag=8)
            gather_norm(1, 0)
            attn_chunk(1, 1, F, quiet_head=1, quiet_tail=2)
            gather_norm(1, 1)
            attn_chunk(1, 2, F, quiet_head=1, quiet_tail=2)
            drain(F)
            gather_norm(1, 2)
            fD = oproj_steps(1, 2, 3)
            attn_chunk(1, 3, fD, quiet_head=1, quiet_tail=3)
            drain(fD)
            gather_norm(1, 3)
            oproj_tail(1, 3)

    nc.compile()
    return nc


def _prep_inputs(hidden_state, attention_mask, Wq, bq, Wk, bk, Wv, bv, Wo, bo):
    h2 = np.ascontiguousarray(
        np.asarray(hidden_state, dtype=np.float32).reshape(BS, D).T
    ).astype(BF16_NP)
    maskT = np.ascontiguousarray(
        np.asarray(attention_mask, dtype=np.float32).reshape(B, S).T)
    selm = np.zeros((33, P), dtype=BF16_NP)
    selm[0, 0:HD] = 1
    selm[32, HD:P] = 1
    in_maps = []
    for c in range(NCORES):
        sl = slice(c * P, (c + 1) * P)
        in_maps.append({
            "hT": h2,
            "wq": np.ascontiguousarray(np.asarray(Wq)[sl, :].T).astype(BF16_NP),
            "wk": np.ascontiguousarray(np.asarray(Wk)[sl, :].T).astype(BF16_NP),
            "wv": np.ascontiguousarray(np.asarray(Wv)[sl, :].T).astype(BF16_NP),
            "wo": np.ascontiguousarray(np.asarray(Wo)[:, sl].T).astype(BF16_NP),
            "bq": np.asarray(bq, dtype=np.float32)[sl].reshape(P, 1),
            "bk": np.asarray(bk, dtype=np.float32)[sl].reshape(P, 1),
            "bv": np.asarray(bv, dtype=np.float32)[sl].reshape(P, 1),
            "bo": np.asarray(bo, dtype=np.float32)[sl].reshape(P, 1),
            "maskT": maskT,
            "sel": selm,
        })
    return in_maps


def kernel(**inputs) -> np.ndarray:
    if "nc" not in _CACHE:
        _CACHE["nc"] = _build()
    nc = _CACHE["nc"]
    in_maps = _prep_inputs(**inputs)
    res = bass_utils.run_bass_kernel_spmd(
        nc, in_maps, core_ids=list(range(NCORES)))
    outT = res.results[0]["outT"].astype(np.float32)
    for c in range(1, NCORES):
        outT += res.results[c]["outT"].astype(np.float32)
    out = np.ascontiguousarray(outT.T).reshape(B, S, D)
    out += np.asarray(inputs["bo"], dtype=np.float32)
    return out.astype(np.float32)
